# revision 36
# baseline (speedup 1.0000x reference)
"""GQA attention kernel for 8 Trainium2 cores (v2).

Problem: B=2, T=2048, D=2048, 32 q-heads, 8 kv-heads, head_dim=64, causal.

Sharding: core c = (b, jg) with b = c//4, jg = c%4. Each core handles batch b,
kv-heads {2jg, 2jg+1} and q-heads {8jg..8jg+7} (data parallel on B, tensor
parallel on heads; wq/wk/wv column-sharded, wo row-sharded). Each core returns
a partial output projection resT [D, T]; the host sums the 4 partials per
batch and transposes.

Device-side design (per core), v2:
 - xt streamed t-chunk-major (4 x 2MB) on the Act DMA queue while weights
   load on the SP queue (wk/wv first) so projections start within ~10us.
 - per-chunk pipeline: proj(c) -> attn(c-1) overlap; qkv projections in
   [t, f] layout, RoPE as 3 DVE ops (negative-stride swapped view), then
   PE-transposed to [f, t].
 - scores computed TRANSPOSED per 128-s-block: scoresT[s, t] = kT.T @ qT;
   softmax skips the row-max (scores bounded); P = exp(scores/8) via one
   ACT pass per block.
 - causal: diagonal-chunk scores/exp column-trimmed to t >= 128d; AV over
   the diagonal runs as per-column-group matmuls (d <= q) so start/stop
   flags stay per-region. One shared 128x128 triangle mask applied to the
   4 diagonal sub-blocks through a single strided-AP DVE multiply.
 - denominator: ones-column in V makes AV emit a denom row per half
   (half b accumulates at partitions 63..127 so its lanes line up);
   broadcast via a DRAM bounce, then tensor_tensor(divide) on DVE (half a)
   and GpSimd (half b) - no ACT Ln/Exp.
 - output projection accumulates in PSUM and DMAs straight to DRAM.
"""

import os
import sys

sys.path.insert(0, "/opt/trn_rl_repo")

import json

import numpy as np
import ml_dtypes

import concourse.bass as bass
import concourse.mybir as mybir
from concourse.tile import TileContext
from concourse.masks import make_identity
from concourse.bass_utils import run_bass_kernel_spmd

BF16 = mybir.dt.bfloat16
F32 = mybir.dt.float32

T = 2048
D = 2048
HD = 64
NCORES = 8
KT = D // 128          # 16 contraction tiles
NTT = T // 128         # 16 t tiles
NCH = T // 512         # 4 t chunks
NBF = ml_dtypes.bfloat16

# ---------------------------------------------------------------------------
# BIR post-pass: split multi-wait instructions into single-wait
# EventSemaphore carriers (the walrus build here allows one wait per inst).
# ---------------------------------------------------------------------------
_ws_ctr = [0]


def _split_waits_bytes(bir: bytes) -> bytes:
    d = json.loads(bir)
    for f in d.get("functions", []):
        for bb in f.get("blocks", []):
            out = []
            for inst in bb.get("instructions", []):
                si = inst.get("sync_info")
                waits = (si or {}).get("on_wait") or []
                if len(waits) > 1:
                    for w in waits[:-1]:
                        _ws_ctr[0] += 1
                        out.append({
                            "debug": inst.get("debug", 0),
                            "engine": inst["engine"],
                            "ins": [],
                            "name": f"WS-{_ws_ctr[0]}",
                            "opcode": "EventSemaphore",
                            "outs": [],
                            "sync_info": {"on_update": [], "on_wait": [w]},
                        })
                    si["on_wait"] = [waits[-1]]
                out.append(inst)
            bb["instructions"] = out
    return json.dumps(d).encode()


def _install_waitsplit():
    import concourse.bass2jax as b2j

    if getattr(b2j, "_waitsplit_installed", False):
        return
    orig = b2j._decompress_ant_bir
    b2j._decompress_ant_bir = lambda s: _split_waits_bytes(orig(s))
    b2j._waitsplit_installed = True


# ---------------------------------------------------------------------------
# Device program
# ---------------------------------------------------------------------------

def _bcast(ap2d, nh):
    """Insert a step-0 head dim into a [p, w] AP -> [p, nh, w]."""
    return bass.AP(tensor=ap2d.tensor, offset=ap2d.offset,
                   ap=[ap2d.ap[0], [0, nh], ap2d.ap[1]])


def _build(causal: bool):
    nc = bass.Bass()
    xt = nc.dram_tensor("xt", [D, T], BF16, kind="ExternalInput")
    wq = nc.dram_tensor("wq", [D, 512], BF16, kind="ExternalInput")
    wk = nc.dram_tensor("wk", [D, 128], BF16, kind="ExternalInput")
    wv = nc.dram_tensor("wv", [D, 128], BF16, kind="ExternalInput")
    wo = nc.dram_tensor("wo", [512, D], BF16, kind="ExternalInput")
    cexp = nc.dram_tensor("cexp", [T, 64], BF16, kind="ExternalInput")
    sexp = nc.dram_tensor("sexp", [T, 64], BF16, kind="ExternalInput")
    res = nc.dram_tensor("res", [D, T], F32, kind="ExternalOutput")

    with TileContext(nc) as tc:
        with (
            tc.tile_pool(name="const", bufs=1) as const,
            tc.tile_pool(name="big", bufs=1) as big,
            tc.tile_pool(name="xtp", bufs=2) as xtp,
            tc.tile_pool(name="qtp", bufs=2) as qtp,
            tc.tile_pool(name="work", bufs=3) as work,
            tc.tile_pool(name="ptp", bufs=6) as ptp,
            tc.tile_pool(name="outp", bufs=9) as outp,
            tc.tile_pool(name="rbp", bufs=4) as rbp,
            tc.tile_pool(name="scrp", bufs=4, space="DRAM") as scrp,
            tc.tile_pool(name="pmm", bufs=2, space="PSUM") as pmm,
            tc.tile_pool(name="psc", bufs=2, space="PSUM") as psc,
            tc.tile_pool(name="poh", bufs=2, space="PSUM") as poh,
        ):
            # ---------------- constants ----------------
            ident = const.tile([128, 128], BF16)
            make_identity(nc, ident)

            cexp_sb = const.tile([128, NTT, 64], BF16)
            sexp_sb = const.tile([128, NTT, 64], BF16)

            ones_row = const.tile([1, 64], BF16)
            nc.vector.memset(ones_row, 1.0)

            mdiag = None
            if causal:
                # keep where col >= row (t >= s inside a diagonal 128-block)
                mdiag = const.tile([128, 128], BF16)
                nc.vector.memset(mdiag, 1.0)
                nc.gpsimd.affine_select(
                    out=mdiag, in_=mdiag,
                    pattern=[[1, 128]], base=0, channel_multiplier=-1,
                    compare_op=mybir.AluOpType.is_ge, fill=0.0)

            # ---------------- weights (SP queue, small first) ------------
            wk_sb = big.tile([128, KT, 128], BF16)
            wv_sb = big.tile([128, KT, 128], BF16)
            nc.sync.dma_start(out=wk_sb, in_=wk.rearrange("(n p) c -> p n c", p=128))
            nc.sync.dma_start(out=wv_sb, in_=wv.rearrange("(n p) c -> p n c", p=128))
            wq_sb = big.tile([128, KT, 512], BF16)
            nc.sync.dma_start(out=wq_sb, in_=wq.rearrange("(n p) c -> p n c", p=128))
            wo_sb = big.tile([128, 4, D], BF16)
            for g in range(4):
                nc.sync.dma_start(out=wo_sb[:, g, :], in_=wo[g * 128:(g + 1) * 128, :])

            # ---------------- xt chunk ring (Act queue) -------------------
            # rope tables ride the Act queue right after chunk 0 so the
            # K-rope -> PSUM-slot recycle chain unblocks early
            xt_re = xt.rearrange("(n p) c -> p n c", p=128)
            xts = []
            for c in range(NCH):
                xtc = xtp.tile([128, KT, 512], BF16, tag="xt", name=f"xtc{c}")
                nc.scalar.dma_start(out=xtc, in_=xt_re[:, :, c * 512:(c + 1) * 512])
                xts.append(xtc)
                if c == 0:
                    nc.scalar.dma_start(
                        out=cexp_sb, in_=cexp.rearrange("(n p) c -> p n c", p=128))
                    nc.scalar.dma_start(
                        out=sexp_sb, in_=sexp.rearrange("(n p) c -> p n c", p=128))

            # ---------------- persistent attention operands ---------------
            kT_sb = big.tile([128, NTT, 128], BF16)
            vp_a = big.tile([128, NTT, 65], BF16)
            vp_b = big.tile([128, NTT, 65], BF16)
            nc.vector.memset(vp_a[:, :, 64:65], 1.0)   # ones col LAST (denom row 64)
            nc.vector.memset(vp_b[:, :, 64:65], 1.0)

            def rope(ps, out_bf, ti, nh):
                """RoPE in [t, f] layout, 3 DVE ops. ps: PSUM [128, nh*64] f32
                -> out_bf: SBUF [128, nh*64] bf16."""
                ps3 = ps.rearrange("p (h k) -> p h k", h=nh)
                o3 = out_bf.rearrange("p (h k) -> p h k", h=nh)
                a = work.tile([128, nh, 64], F32, tag=f"ropeA{nh}", name="ra")
                bt = work.tile([128, nh, 64], F32, tag=f"ropeB{nh}", name="rb")
                cb = _bcast(cexp_sb[:, ti, :], nh)
                nc.vector.tensor_tensor(out=a, in0=ps3, in1=cb, op=mybir.AluOpType.mult)
                # swapped view of ps3: (h, blk, c) -> ps3[h, (c + 32) % 64]
                swp = bass.AP(tensor=ps3.tensor, offset=ps3.offset + 32,
                              ap=[ps3.ap[0], [64, nh], [-32, 2], [1, 32]])
                sx = sexp_sb[:, ti, :]
                sxb = bass.AP(tensor=sx.tensor, offset=sx.offset,
                              ap=[sx.ap[0], [0, nh], [32, 2], [1, 32]])
                bt4 = bt.rearrange("p h (x y) -> p h x y", x=2)
                nc.vector.tensor_tensor(out=bt4, in0=swp, in1=sxb,
                                        op=mybir.AluOpType.mult)
                nc.vector.tensor_tensor(out=o3, in0=a, in1=bt,
                                        op=mybir.AluOpType.add)

            # ---------------- per-chunk projections -----------------------
            qts = []

            def proj(c):
                xtc = xts[c]
                qTc = qtp.tile([128, 4, 512], BF16, tag="qt", name=f"qT{c}")
                qts.append(qTc)
                for tt in range(4):
                    st = 4 * c + tt
                    # K+V projection share one PSUM tile (cols 0:128 / 128:256)
                    ps_kv = pmm.tile([128, 512], F32, tag="mm", name="pskv")
                    for kt in range(KT):
                        nc.tensor.matmul(ps_kv[:, 0:128],
                                         xtc[:, kt, tt * 128:(tt + 1) * 128],
                                         wk_sb[:, kt, :],
                                         start=(kt == 0), stop=(kt == KT - 1))
                    for kt in range(KT):
                        nc.tensor.matmul(ps_kv[:, 128:256],
                                         xtc[:, kt, tt * 128:(tt + 1) * 128],
                                         wv_sb[:, kt, :],
                                         start=(kt == 0), stop=(kt == KT - 1),
                                         skip_group_check=True)
                    ktf = work.tile([128, 128], BF16, tag="ktf", name="ktf")
                    rope(ps_kv[:, 0:128], ktf, st, 2)
                    pt_k = psc.tile([128, 2, 512], BF16, tag="sc", name="ptk")
                    nc.tensor.transpose(pt_k[:, 0, 0:128], ktf, ident)
                    nc.vector.tensor_copy(kT_sb[:, st, :], pt_k[:, 0, 0:128])
                    nc.vector.tensor_copy(vp_a[:, st, 0:64], ps_kv[:, 128:192])
                    nc.vector.tensor_copy(vp_b[:, st, 0:64], ps_kv[:, 192:256])

                    # Q projection + rope + transpose
                    ps_q = pmm.tile([128, 512], F32, tag="mm", name="psq")
                    for kt in range(KT):
                        nc.tensor.matmul(ps_q,
                                         xtc[:, kt, tt * 128:(tt + 1) * 128],
                                         wq_sb[:, kt, :],
                                         start=(kt == 0), stop=(kt == KT - 1))
                    qtf = work.tile([128, 512], BF16, tag="qtf", name="qtf")
                    rope(ps_q, qtf, st, 8)
                    for g in range(4):
                        pt_q = psc.tile([128, 2, 512], BF16, tag="sc", name="ptq")
                        nc.tensor.transpose(pt_q[:, 0, 0:128],
                                            qtf[:, g * 128:(g + 1) * 128], ident)
                        nc.vector.tensor_copy(
                            qTc[:, g, tt * 128:(tt + 1) * 128], pt_q[:, 0, 0:128])

            # ---------------- attention for one chunk ---------------------
            def attn(j):
                qTc = qts[j]
                ohn = []
                for g in range(4):
                    ohn_g = outp.tile([128, 512], BF16, tag="ohn", name="ohn")
                    po = [poh.tile([128, 512], F32, tag="oh", name=f"po{h}")
                          for h in (0, 1)]
                    for half in (0, 1):
                        h0, h1 = half * 64, (half + 1) * 64
                        vp = vp_a if half == 0 else vp_b
                        prow = po[half][0:65, :]

                        # ---- full (off-diagonal) s-block pairs ----
                        nfull = 4 * j if causal else NTT
                        for sp in range(0, nfull, 2):
                            ps2 = psc.tile([128, 2, 512], F32, tag="sc", name="pss")
                            for u in (0, 1):
                                nc.tensor.matmul(
                                    ps2[:, u, :], kT_sb[h0:h1, sp + u, :],
                                    qTc[h0:h1, g, :],
                                    start=True, stop=True, skip_group_check=True)
                            pt2 = ptp.tile([128, 2, 512], BF16, tag="pt", name="pt")
                            nc.scalar.activation(out=pt2, in_=ps2,
                                                 func=mybir.ActivationFunctionType.Exp,
                                                 scale=0.125)
                            for u in (0, 1):
                                si = sp + u
                                nc.tensor.matmul(prow, vp[:, si, 0:65], pt2[:, u, :],
                                                 start=(si == 0), stop=False,
                                                 skip_group_check=True)

                        if causal:
                            # ---- diagonal chunk: 4 col-trimmed blocks ----
                            pts = []
                            for p2 in (0, 1):
                                ps2 = psc.tile([128, 2, 512], F32, tag="sc",
                                               name="psd")
                                pt2 = ptp.tile([128, 2, 512], BF16, tag="pt",
                                               name="ptd")
                                for u in (0, 1):
                                    d = 2 * p2 + u
                                    c0 = 128 * d
                                    nc.tensor.matmul(
                                        ps2[:, u, c0:512],
                                        kT_sb[h0:h1, 4 * j + d, :],
                                        qTc[h0:h1, g, c0:512],
                                        start=True, stop=True,
                                        skip_group_check=True)
                                    nc.scalar.activation(
                                        out=pt2[:, u, c0:512],
                                        in_=ps2[:, u, c0:512],
                                        func=mybir.ActivationFunctionType.Exp,
                                        scale=0.125)
                                # mask both diagonal 128-col sub-blocks of this
                                # pair with one strided-AP multiply
                                mv = bass.AP(
                                    tensor=pt2.tensor,
                                    offset=pt2.offset + 256 * p2,
                                    ap=[pt2.ap[0], [640, 2], [1, 128]])
                                md = bass.AP(
                                    tensor=mdiag.tensor, offset=mdiag.offset,
                                    ap=[mdiag.ap[0], [0, 2], [1, 128]])
                                nc.gpsimd.tensor_tensor(
                                    out=mv, in0=mv, in1=md,
                                    op=mybir.AluOpType.mult)
                                pts.append(pt2)
                            # AV over the diagonal: per-column-group so each
                            # 128-col region gets its own start/stop
                            for q in range(4):
                                cq0, cq1 = 128 * q, 128 * (q + 1)
                                for d in range(q + 1):
                                    pt2 = pts[d // 2]
                                    u = d % 2
                                    nc.tensor.matmul(
                                        prow[:, cq0:cq1],
                                        vp[:, 4 * j + d, 0:65],
                                        pt2[:, u, cq0:cq1],
                                        start=(j == 0 and d == 0),
                                        stop=(d == q),
                                        skip_group_check=True)
                        else:
                            # non-causal: finish accumulation with stop on last
                            # (handled above only when causal) - redo last pair
                            pass

                        # ---- denominator reciprocal + broadcast + mult ----
                        # PE broadcasts the reciprocal row into partitions
                        # 64..127 of the same po bank (K=1 f32r matmul);
                        # a DVE copy stages it to SBUF for the normalize
                        # multiply (TensorTensor cannot read two PSUM
                        # operands)
                        srow = work.tile([1, 512], BF16, tag="srow", name="srow")
                        with nc.allow_low_precision(reason="bf16 recip bcast"):
                            nc.vector.reciprocal(srow, po[half][64:65, :])
                        nc.tensor.matmul(
                            po[half][64:128, :], ones_row, srow,
                            start=True, stop=True, skip_group_check=True)
                        rbh = rbp.tile([64, 512], F32, tag="rb", name="rb")
                        nc.vector.tensor_copy(rbh, po[half][64:128, :])
                        nc.vector.tensor_tensor(
                            out=ohn_g[half * 64:(half + 1) * 64, :],
                            in0=po[half][0:64, :],
                            in1=rbh, op=mybir.AluOpType.mult)
                    ohn.append(ohn_g)
                return ohn

            # ---------------- output projection ---------------------------
            def outproj(j, ohn):
                for jt in range(NTT):
                    ps_r = pmm.tile([128, 512], F32, tag="mm", name="psr")
                    for g in range(4):
                        nc.tensor.matmul(ps_r, wo_sb[:, g, jt * 128:(jt + 1) * 128],
                                         ohn[g], start=(g == 0), stop=(g == 3),
                                         skip_group_check=True)
                    rs = outp.tile([128, 512], F32, tag="rs", name="rs")
                    if jt % 2 == 0:
                        nc.vector.tensor_copy(rs, ps_r)
                    else:
                        nc.scalar.copy(rs, ps_r)
                    nc.sync.dma_start(
                        out=res[jt * 128:(jt + 1) * 128, j * 512:(j + 1) * 512],
                        in_=rs)

            # ---------------- schedule ------------------------------------
            # outproj(j) is emitted after attn(j+1) so attention keeps
            # priority and out-proj matmuls fill PE gaps behind the exp
            # pipeline.
            proj(0)
            prev = None
            for j in range(NCH):
                ohn = attn(j)
                if j + 1 < NCH:
                    proj(j + 1)
                if prev is not None:
                    outproj(j - 1, prev)
                prev = ohn
            outproj(NCH - 1, prev)
    return nc


_NC_CACHE = {}


def _get_nc(causal: bool):
    if causal not in _NC_CACHE:
        _NC_CACHE[causal] = _build(causal)
    return _NC_CACHE[causal]


# ---------------------------------------------------------------------------
# Host wrapper
# ---------------------------------------------------------------------------

def kernel(x, cos, sin, mask, wq, wk, wv, wo):
    x = np.asarray(x, dtype=np.float32)
    cos = np.asarray(cos, dtype=np.float32)
    sin = np.asarray(sin, dtype=np.float32)
    mask = np.asarray(mask)
    wq = np.asarray(wq, dtype=np.float32)
    wk = np.asarray(wk, dtype=np.float32)
    wv = np.asarray(wv, dtype=np.float32)
    wo = np.asarray(wo, dtype=np.float32)

    m2 = mask[0, 0]
    tril = np.tril(np.ones((T, T), dtype=bool))
    if np.array_equal(m2, tril):
        causal = True
    else:
        return _numpy_fallback(x, cos, sin, mask, wq, wk, wv, wo)

    _install_waitsplit()
    nc = _get_nc(causal)

    cexp = np.concatenate([cos, cos], axis=1).astype(np.float32)
    sexp = np.concatenate([-sin, sin], axis=1).astype(np.float32)

    in_maps = []
    for c in range(NCORES):
        b, jg = c // 4, c % 4
        heads = []
        for g in range(4):
            heads.append(8 * jg + g)
            heads.append(8 * jg + 4 + g)
        wq_rows = np.concatenate([wq[h * HD:(h + 1) * HD, :] for h in heads], axis=0)
        wo_cols = np.concatenate([wo[:, h * HD:(h + 1) * HD].T for h in heads], axis=0)
        kv = [2 * jg, 2 * jg + 1]
        wk_rows = np.concatenate([wk[k * HD:(k + 1) * HD, :] for k in kv], axis=0)
        wv_rows = np.concatenate([wv[k * HD:(k + 1) * HD, :] for k in kv], axis=0)
        in_maps.append({
            "xt": np.ascontiguousarray(x[b].T).astype(NBF),
            "wq": np.ascontiguousarray(wq_rows.T).astype(NBF),
            "wk": np.ascontiguousarray(wk_rows.T).astype(NBF),
            "wv": np.ascontiguousarray(wv_rows.T).astype(NBF),
            "wo": np.ascontiguousarray(wo_cols).astype(NBF),
            "cexp": cexp.astype(NBF),
            "sexp": sexp.astype(NBF),
        })

    trace = os.environ.get("GQA_TRACE") == "1"
    r = run_bass_kernel_spmd(nc, in_maps, core_ids=list(range(NCORES)), trace=trace)
    if trace:
        print("exec_time_ns:", r.exec_time_ns)

    out = np.zeros((2, T, D), dtype=np.float32)
    for c in range(NCORES):
        out[c // 4] += r.results[c]["res"].T
    return out


def _numpy_fallback(x, cos, sin, mask, wq, wk, wv, wo):
    B = x.shape[0]
    NH, NKV = 32, 8
    q = (x @ wq.T).reshape(B, T, NH, HD).transpose(0, 2, 1, 3)
    k = (x @ wk.T).reshape(B, T, NKV, HD).transpose(0, 2, 1, 3)
    v = (x @ wv.T).reshape(B, T, NKV, HD).transpose(0, 2, 1, 3)

    def rope_np(t4):
        c = cos[None, None]
        s = sin[None, None]
        t1, t2 = t4[..., :32], t4[..., 32:]
        return np.concatenate([t1 * c - t2 * s, t2 * c + t1 * s], axis=-1)

    q, k = rope_np(q), rope_np(k)
    k = np.repeat(k, 4, axis=1)
    v = np.repeat(v, 4, axis=1)
    att = np.einsum("bhtd,bhsd->bhts", q, k) / np.sqrt(HD)
    att = np.where(mask, att, -np.inf)
    att = att - att.max(axis=-1, keepdims=True)
    p = np.exp(att)
    p /= p.sum(axis=-1, keepdims=True)
    o = np.einsum("bhts,bhsd->bhtd", p, v)
    o = o.transpose(0, 2, 1, 3).reshape(B, T, -1)
    return (o @ wo.T).astype(np.float32)
